# revision 19
# baseline (speedup 1.0000x reference)
"""Trainium2 Bass kernel: BasicMultiheadAttention (B=2, S=2048, D=1024, H=16).

Sharding: tensor-parallel over heads (core c owns heads 2c, 2c+1 for both
batches) for QKV + attention; token-parallel for the output projection
(each core computes ALL 1024 output dims for its 1/8 slice of tokens),
connected by a per-half-batch AllToAll of ctx^T (8x less wire than the
AllGather + column-sharded scheme).

Softmax exp is split across two engines: ACT does true exp (PSUM->SBUF f16),
DVE does a Schraudolph bit-trick exp (int16(A*x+B) bitcast as f16) for a
subset of key tiles.  The denominator rides in the PV matmul as a ones
column (row 64 of ctx psum); normalization is recip_approx_fast on the
[1,512] denom row + an e0-weighted f32r matmul that broadcasts the
reciprocal across partitions + one DVE multiply.

QKV for batch 1 is interleaved into batch 0's attention so the PE stays
dense while ACT/DVE chew exp.
"""

import numpy as np

B, S, D, H = 2, 2048, 1024, 16
DH = D // H  # 64
NCORES = 8
HPC = H // NCORES  # heads per core = 2
SQ = B * S  # 4096 tokens
NKT = D // 128  # 8 contraction k-tiles over D
KT_S = S // 128  # 16 key tiles per batch
QC_S = S // 512  # 4 query chunks of 512 per batch
NHB = 4  # half-batches (a2a granularity): 1024 tokens each
TOKCH = 128  # tokens per core per half-batch after AllToAll

# Schraudolph exp-as-f16-bits: bits = round(1024*(x*0.125*log2e + 15 - sigma))
_SCH_A = 0.125 * np.log2(np.e) * 1024.0
_SCH_B = (15.0 - 0.0430) * 1024.0
# h1 exp goes to DVE for these key tiles (h0 exp always on ACT; plus h1 on
# ACT outside this range).  12 of 16 -> ACT:20 DVE:12 units per qc.
_DVE_H1_KT = set(range(3, 15))

_CACHE = {}


def _ensure_axon_hooks():
    """This image's antenv lacks axon_hooks; bass_utils imports it when
    trace=True under axon. Register an equivalent stub backed by the boot
    helper so NTFF profiling works (or degrades gracefully)."""
    import sys
    import types
    try:
        import antenv.axon_hooks  # noqa: F401
        return
    except ImportError:
        pass
    try:
        import antenv
        hook = [None]
        try:
            from trn_agent_boot.trn_boot import _ntff_profile_via_ctypes
            hook[0] = _ntff_profile_via_ctypes("/opt/axon/libaxon_pjrt.so")
        except Exception:
            hook[0] = None
        mod = types.ModuleType("antenv.axon_hooks")
        mod.get_axon_ntff_profile_hook = lambda: hook[0]
        mod.set_axon_ntff_profile_hook = lambda h: hook.__setitem__(0, h)
        sys.modules["antenv.axon_hooks"] = mod
        antenv.axon_hooks = mod
    except Exception:
        pass


_ensure_axon_hooks()


def _build_kernel():
    import concourse.bass as bass  # noqa: F401
    import concourse.mybir as mybir
    import concourse.tile as tile
    from concourse import bacc
    from concourse.masks import make_identity

    f16 = mybir.dt.float16
    f32 = mybir.dt.float32
    f32r = mybir.dt.float32r
    i16 = mybir.dt.int16
    AF = mybir.ActivationFunctionType
    ALU = mybir.AluOpType

    nc = bacc.Bacc(None, num_devices=NCORES)

    # ---- I/O ----
    xT = nc.dram_tensor("xT", [D, SQ], f16, kind="ExternalInput")
    # wq|wk|wv (per-core heads, kt-major) then full Wo (kt-major, 8*1024)
    wpack = nc.dram_tensor("wpack", [128, 3 * D + NKT * D], f16,
                           kind="ExternalInput")
    bpack = nc.dram_tensor("bpack", [128, 4], f32, kind="ExternalInput")
    boful = nc.dram_tensor("boful", [1, D], f16, kind="ExternalInput")
    # token-sharded output: row hb*128+i, col o  (token hb*1024 + core*128 + i)
    yO = nc.dram_tensor("yO", [NHB * 128, D], f32, kind="ExternalOutput")
    dbgCtx = nc.dram_tensor("dbgCtx", [128, SQ], f16, kind="ExternalOutput")

    with tile.TileContext(nc) as tc:
        with (
            tc.tile_pool(name="const", bufs=1) as const,
            tc.tile_pool(name="psSc", bufs=3, space="PSUM") as psSc,
            tc.tile_pool(name="psCtx", bufs=2, space="PSUM") as psCtx,
            tc.tile_pool(name="psR", bufs=1, space="PSUM") as psR,
            tc.tile_pool(name="psO", bufs=2, space="PSUM") as psO,
            tc.tile_pool(name="pP", bufs=4) as pP,
            tc.tile_pool(name="pDen", bufs=2) as pDen,
            tc.tile_pool(name="pCg", bufs=2) as pCg,
            tc.tile_pool(name="pOut", bufs=2) as pOut,
            tc.tile_pool(name="dram", bufs=1, space="DRAM") as dram,
        ):
            # ---- constants / small DMAs first ----
            wpack_sb = const.tile([128, 3 * D + NKT * D], f16)
            nc.sync.dma_start(wpack_sb[:], wpack[:, :])
            bpack_sb = const.tile([128, 4], f32)
            nc.sync.dma_start(bpack_sb[:], bpack[:, :])
            bo_sb = const.tile([128, D], f16)
            nc.vector.memset(bo_sb[:], 0.0)
            nc.sync.dma_start(bo_sb[0:1, :], boful[:, :])
            wq_sb = wpack_sb[:, 0 * D:1 * D]
            wk_sb = wpack_sb[:, 1 * D:2 * D]
            wv_sb = wpack_sb[:, 2 * D:3 * D]
            wo_sb = wpack_sb[:, 3 * D:]  # [128, NKT*D] kt-major
            bq_sb = bpack_sb[:, 0:1]
            bk_sb = bpack_sb[:, 1:2]
            bvt_sb = bpack_sb[:, 2:3]

            ident = const.tile([128, 128], f16)
            make_identity(nc, ident)

            # eh weights: row h*64 ones, rest zero (selects that partition)
            e0 = const.tile([128, 128], f16)
            nc.vector.memset(e0[:], 0.0)
            nc.vector.memset(e0[0:1, :], 1.0)
            e1 = const.tile([128, 128], f16)
            nc.vector.memset(e1[:], 0.0)
            nc.vector.memset(e1[64:65, :], 1.0)
            esel = [e0, e1]

            # reciprocal staging: rows 0/1 get the two heads' denominators,
            # rows 2..127 stay 1.0 so recip_approx_fast stays finite there
            # (the op misbehaves on <128-partition APs).
            rec_st = const.tile([128, 512], f32, name="rec_st")
            nc.vector.memset(rec_st[:], 1.0)
            rec_full = const.tile([128, 512], f32, name="rec_full")
            rec16 = [const.tile([128, 512], f16, name=f"rec16{h}")
                     for h in range(HPC)]
            for h in range(HPC):
                nc.vector.memset(rec16[h][:], 0.0)

            # bo broadcast to all 128 partitions via e0 matmul (one-time)
            bo_bc = const.tile([128, D], f32)
            for half in range(2):
                bo_ps = psO.tile([128, 512], f32, tag="a", name=f"bops_{half}")
                nc.tensor.matmul(
                    bo_ps[:], lhsT=e0[:],
                    rhs=bo_sb[:, half * 512:(half + 1) * 512],
                    start=True, stop=True,
                )
                nc.vector.tensor_copy(bo_bc[:, half * 512:(half + 1) * 512],
                                      bo_ps[:])

            # ---- xT load, segment-major so QKV can start early ----
            xt_sb = const.tile([128, NKT * SQ], f16)
            for seg in range(4):
                t0 = seg * 1024
                for kt in range(NKT):
                    nc.sync.dma_start(
                        xt_sb[:, kt * SQ + t0: kt * SQ + t0 + 1024],
                        xT[kt * 128:(kt + 1) * 128, t0:t0 + 1024],
                    )

            qT_sb = const.tile([128, SQ], f16)
            kT_sb = const.tile([128, SQ], f16)
            vT_sb = const.tile([128, SQ], f16)
            # V with ones column: per (b, head, key-tile) a [128, 65] region
            NREG = B * HPC * KT_S  # 64 regions
            vaug_sb = const.tile([128, NREG * 65], f16)
            ones_cols = vaug_sb.rearrange("p (r c) -> p r c", c=65)[:, :, 64:65]
            nc.vector.memset(ones_cols, 1.0)

            ctxT_sb = const.tile([128, SQ], f16)

            # warmup collective: absorb ncfw first-trigger init early
            wu_loc = dram.tile([NCORES, 2], f16, name="wu_loc")
            nc.sync.dma_start(wu_loc[:], ctxT_sb[0:NCORES, 0:2])
            wu_g = dram.tile([NCORES, 2], f16, name="wu_g")
            nc.gpsimd.collective_compute(
                "AllToAll", mybir.AluOpType.bypass,
                replica_groups=[list(range(NCORES))],
                ins=[wu_loc.opt()], outs=[wu_g.opt()],
            )

            # ---- QKV helpers ----
            def emit_qkv_chunk(b, proj, ncx):
                w_sb, b_sb, dst = (
                    (wq_sb, bq_sb, qT_sb), (wk_sb, bk_sb, kT_sb),
                    (wv_sb, bvt_sb, vT_sb),
                )[proj]
                tok0 = b * S + ncx * 512
                ps = psO.tile([128, 512], f32, tag="a",
                              name=f"qkv_{b}_{proj}_{ncx}")
                for kt in range(NKT):
                    nc.tensor.matmul(
                        ps[:],
                        lhsT=w_sb[:, kt * 128:(kt + 1) * 128],
                        rhs=xt_sb[:, kt * SQ + tok0: kt * SQ + tok0 + 512],
                        start=(kt == 0),
                        stop=(kt == NKT - 1),
                    )
                nc.scalar.activation(dst[:, tok0:tok0 + 512], ps[:],
                                     AF.Identity, bias=b_sb)

            def emit_vaug(b):
                # transpose V^T tiles into [tokens, dims] regions
                for tt in range(KT_S):
                    tok0 = b * S + tt * 128
                    vtr = psO.tile([128, 128], f16, tag="a",
                                   name=f"vtr_{b}_{tt}")
                    nc.tensor.transpose(vtr[:], vT_sb[:, tok0:tok0 + 128],
                                        ident[:])
                    for h in range(HPC):
                        r = (b * HPC + h) * KT_S + tt
                        nc.vector.tensor_copy(
                            vaug_sb[:, r * 65: r * 65 + 64],
                            vtr[:, h * 64:(h + 1) * 64],
                        )

            # ---- attention inner loop ----
            def emit_attention_qc(b, qc):
                q0 = b * S + qc * 512
                ctx_ps = [
                    psCtx.tile([65, 512], f32, tag="ctx",
                               name=f"ctx_{b}_{qc}_{h}")
                    for h in range(HPC)
                ]
                for kt in range(KT_S):
                    k0 = b * S + kt * 128
                    scs = []
                    for h in range(HPC):
                        sc = psSc.tile([128, 512], f32, tag="sc",
                                       name=f"sc_{b}_{qc}_{kt}_{h}")
                        nc.tensor.matmul(
                            sc[:],
                            lhsT=kT_sb[h * 64:(h + 1) * 64, k0:k0 + 128],
                            rhs=qT_sb[h * 64:(h + 1) * 64, q0:q0 + 512],
                            start=True, stop=True,
                            tile_position=(h * 64, 0),
                        )
                        scs.append(sc)
                    p_sb = pP.tile([128, 1024], f16, tag="p",
                                   name=f"p_{b}_{qc}_{kt}")
                    for h in range(HPC):
                        use_dve = (h == 1 and kt in _DVE_H1_KT)
                        if use_dve:
                            nc.vector.tensor_scalar(
                                p_sb.bitcast(i16)[:, h * 512:(h + 1) * 512],
                                scs[h][:], _SCH_A, _SCH_B,
                                ALU.mult, ALU.add,
                            )
                        else:
                            nc.scalar.activation(
                                p_sb[:, h * 512:(h + 1) * 512], scs[h][:],
                                AF.Exp, scale=0.125,
                            )
                    for h in range(HPC):
                        r = (b * HPC + h) * KT_S + kt
                        nc.tensor.matmul(
                            ctx_ps[h][:],
                            lhsT=vaug_sb[:, r * 65:(r + 1) * 65],
                            rhs=p_sb[:, h * 512:(h + 1) * 512],
                            start=(kt == 0),
                            stop=(kt == KT_S - 1),
                        )
                # normalize: denom row 64 -> recip -> broadcast -> multiply
                rps = psR.tile([128, 512], f32, tag="rps", name=f"rps_{b}_{qc}")
                for h in range(HPC):
                    nc.vector.tensor_copy(rec_st[h * 64:h * 64 + 1, :],
                                          ctx_ps[h][64:65, :])
                nc.vector.reciprocal_approx_fast(rec_full[:], rec_st[:])
                for h in range(HPC):
                    nc.vector.tensor_copy(rec16[h][h * 64:h * 64 + 1, :],
                                          rec_full[h * 64:h * 64 + 1, :])
                    nc.tensor.matmul(
                        rps[h * 64:(h + 1) * 64, :],
                        lhsT=esel[h][:, 0:64],
                        rhs=rec16[h][:],
                        start=True, stop=True,
                    )
                for h in range(HPC):
                    rps_sb = pDen.tile([64, 512], f16, tag="d",
                                       name=f"rd_{b}_{qc}_{h}")
                    nc.scalar.copy(rps_sb[:], rps[h * 64:(h + 1) * 64, :])
                    nc.vector.tensor_tensor(
                        ctxT_sb[h * 64:(h + 1) * 64, q0:q0 + 512],
                        ctx_ps[h][0:64, :], rps_sb[:], ALU.mult,
                    )

            # ---- AllToAll + token-sharded output projection ----
            a2a_outs = {}

            def emit_a2a(hb):
                t0 = hb * 1024
                a_in = dram.tile([NCORES * 128, TOKCH], f16,
                                 name=f"a2a_in_{hb}")
                for j in range(NCORES):
                    nc.sync.dma_start(
                        a_in[j * 128:(j + 1) * 128, :],
                        ctxT_sb[:, t0 + j * TOKCH: t0 + (j + 1) * TOKCH],
                    )
                a_out = dram.tile([NCORES * 128, TOKCH], f16,
                                  name=f"a2a_out_{hb}")
                nc.gpsimd.collective_compute(
                    "AllToAll", mybir.AluOpType.bypass,
                    replica_groups=[list(range(NCORES))],
                    ins=[a_in.opt()], outs=[a_out.opt()],
                )
                a2a_outs[hb] = a_out

            def emit_outproj(hb):
                a_out = a2a_outs[hb]
                cg = pCg.tile([128, NCORES, TOKCH], f16, tag="cg",
                              name=f"cg_{hb}")
                for k in range(NCORES):
                    nc.sync.dma_start(cg[:, k, :],
                                      a_out[k * 128:(k + 1) * 128, :])
                pos = [psO.tile([128, 512], f32, tag="a", name=f"po_{hb}_{i}")
                       for i in range(2)]
                for half in range(2):
                    for k in range(NKT):
                        nc.tensor.matmul(
                            pos[half][:],
                            lhsT=cg[:, k, :],
                            rhs=wo_sb[:, k * D + half * 512:
                                      k * D + half * 512 + 512],
                            start=(k == 0),
                            stop=(k == NKT - 1),
                        )
                out_sb = pOut.tile([128, D], f32, tag="os", name=f"os_{hb}")
                for half in range(2):
                    nc.vector.tensor_tensor(
                        out_sb[:, half * 512:(half + 1) * 512],
                        pos[half][:],
                        bo_bc[:, half * 512:(half + 1) * 512],
                        ALU.add,
                    )
                nc.sync.dma_start(yO[hb * 128:(hb + 1) * 128, :], out_sb[:])

            # ---- schedule ----
            # QKV b0 fully, then b0 attention with b1's QKV interleaved.
            for proj in range(3):
                for ncx in range(QC_S):
                    emit_qkv_chunk(0, proj, ncx)
            emit_vaug(0)

            # b1 QKV chunks to interleave: (proj, ncx) pairs, 3 per b0-qc
            b1_chunks = [(p, n) for p in range(3) for n in range(QC_S)]
            pending_outproj = []
            for qc in range(QC_S):
                emit_attention_qc(0, qc)
                if qc % 2 == 1:
                    hb = qc // 2
                    emit_a2a(hb)
                    pending_outproj.append(hb)
                for _ in range(3):
                    if b1_chunks:
                        p_, n_ = b1_chunks.pop(0)
                        emit_qkv_chunk(1, p_, n_)
                if len(pending_outproj) > 1:
                    emit_outproj(pending_outproj.pop(0))
            emit_vaug(1)
            for qc in range(QC_S):
                emit_attention_qc(1, qc)
                if qc % 2 == 1:
                    hb = 2 + qc // 2
                    emit_a2a(hb)
                    pending_outproj.append(hb)
                if len(pending_outproj) > 1:
                    emit_outproj(pending_outproj.pop(0))
            for hb in pending_outproj:
                emit_outproj(hb)
            nc.sync.dma_start(dbgCtx[:, :], ctxT_sb[:])

    nc.finalize()
    return nc


def kernel(x, Wq, Wk, Wv, bq, bk, bv, Wo, bo):
    from concourse.bass_utils import run_bass_kernel_spmd

    if "nc" not in _CACHE:
        _CACHE["nc"] = _build_kernel()
    nc = _CACHE["nc"]

    # host-side prep
    xTh = np.ascontiguousarray(
        x.astype(np.float32).transpose(2, 0, 1).reshape(D, SQ)
    ).astype(np.float16)

    def pack_w(Wslice):
        # [D, M] -> [128, NKT*M] kt-major: out[p, kt*M+m] = Wslice[kt*128+p, m]
        M = Wslice.shape[1]
        return np.ascontiguousarray(
            Wslice.reshape(NKT, 128, M).transpose(1, 0, 2).reshape(128, NKT * M)
        ).astype(np.float16)

    wo_full = pack_w(Wo)  # [128, 8*1024], same on all cores
    bo_row = np.ascontiguousarray(bo.reshape(1, D)).astype(np.float16)

    in_maps = []
    for c in range(NCORES):
        hA, hB = HPC * c, HPC * c + 1
        wq_c = pack_w(np.concatenate([Wq[hA], Wq[hB]], axis=1))
        wk_c = pack_w(np.concatenate([Wk[hA], Wk[hB]], axis=1))
        wv_c = pack_w(np.concatenate([Wv[hA], Wv[hB]], axis=1))
        wpack_c = np.ascontiguousarray(
            np.concatenate([wq_c, wk_c, wv_c, wo_full], axis=1))
        bq_c = np.concatenate([bq[hA], bq[hB]]).reshape(128, 1)
        bk_c = np.concatenate([bk[hA], bk[hB]]).reshape(128, 1)
        bv_c = np.concatenate([bv[hA], bv[hB]]).reshape(128, 1)
        bpack_c = np.ascontiguousarray(
            np.concatenate([bq_c, bk_c, bv_c, np.zeros((128, 1))],
                           axis=1)).astype(np.float32)
        in_maps.append({"xT": xTh, "wpack": wpack_c, "bpack": bpack_c,
                        "boful": bo_row})

    res = run_bass_kernel_spmd(nc, in_maps, core_ids=list(range(NCORES)))
    _CACHE["last_result"] = res
    # assemble: core c's yO row hb*128+i is token (b=hb//2,
    # s=(hb%2)*1024 + c*128 + i), all 1024 output dims
    out = np.empty((B, S, D), dtype=np.float32)
    for c in range(NCORES):
        yo = res.results[c]["yO"]  # [NHB*128, D]
        for hb in range(NHB):
            b = hb // 2
            s0 = (hb % 2) * 1024 + c * 128
            out[b, s0:s0 + 128, :] = yo[hb * 128:(hb + 1) * 128, :]
    return out


# revision 25
# speedup vs baseline: 1.1538x; 1.1538x over previous
"""Trainium2 Bass kernel: BasicMultiheadAttention (B=2, S=2048, D=1024, H=16).

Sharding: tensor-parallel over heads (core c owns heads 2c, 2c+1 for both
batches) for QKV + attention; token-parallel for the output projection
(each core computes ALL 1024 output dims for its 1/8 slice of tokens),
connected by a per-half-batch AllToAll of ctx^T (8x less wire than the
AllGather + column-sharded scheme).

Softmax exp is split across two engines: ACT does true exp (PSUM->SBUF f16),
DVE does a Schraudolph bit-trick exp (int16(A*x+B) bitcast as f16) for a
subset of key tiles.  The denominator rides in the PV matmul as a ones
column (row 64 of ctx psum); normalization is recip_approx_fast on the
[1,512] denom row + an e0-weighted f32r matmul that broadcasts the
reciprocal across partitions + one DVE multiply.

QKV for batch 1 is interleaved into batch 0's attention so the PE stays
dense while ACT/DVE chew exp.
"""

import numpy as np

B, S, D, H = 2, 2048, 1024, 16
DH = D // H  # 64
NCORES = 8
HPC = H // NCORES  # heads per core = 2
SQ = B * S  # 4096 tokens
NKT = D // 128  # 8 contraction k-tiles over D
KT_S = S // 128  # 16 key tiles per batch
QC_S = S // 512  # 4 query chunks of 512 per batch
NHB = 4  # half-batches (a2a granularity): 1024 tokens each
TOKCH = 128  # tokens per core per half-batch after AllToAll

# Schraudolph exp-as-f16-bits: bits = round(1024*(x*0.125*log2e + 15 - sigma))
_SCH_A = 0.125 * np.log2(np.e) * 1024.0
_SCH_B = (15.0 - 0.0430) * 1024.0
# h1 exp goes to DVE for these key tiles (h0 exp always on ACT; plus h1 on
# ACT outside this range).  12 of 16 -> ACT:20 DVE:12 units per qc.
_DVE_H1_KT = set(range(3, 15))

_CACHE = {}


def _ensure_axon_hooks():
    """This image's antenv lacks axon_hooks; bass_utils imports it when
    trace=True under axon. Register an equivalent stub backed by the boot
    helper so NTFF profiling works (or degrades gracefully)."""
    import sys
    import types
    try:
        import antenv.axon_hooks  # noqa: F401
        return
    except ImportError:
        pass
    try:
        import antenv
        hook = [None]
        try:
            from trn_agent_boot.trn_boot import _ntff_profile_via_ctypes
            hook[0] = _ntff_profile_via_ctypes("/opt/axon/libaxon_pjrt.so")
        except Exception:
            hook[0] = None
        mod = types.ModuleType("antenv.axon_hooks")
        mod.get_axon_ntff_profile_hook = lambda: hook[0]
        mod.set_axon_ntff_profile_hook = lambda h: hook.__setitem__(0, h)
        sys.modules["antenv.axon_hooks"] = mod
        antenv.axon_hooks = mod
    except Exception:
        pass


_ensure_axon_hooks()


def _build_kernel():
    import concourse.bass as bass  # noqa: F401
    import concourse.mybir as mybir
    import concourse.tile as tile
    from concourse import bacc
    from concourse.masks import make_identity

    f16 = mybir.dt.float16
    f32 = mybir.dt.float32
    f32r = mybir.dt.float32r
    i16 = mybir.dt.int16
    AF = mybir.ActivationFunctionType
    ALU = mybir.AluOpType

    nc = bacc.Bacc(None, num_devices=NCORES)

    # ---- I/O ----
    xT = nc.dram_tensor("xT", [D, SQ], f16, kind="ExternalInput")
    # wq|wk|wv (per-core heads, kt-major) then per-core Wo slice (kt-major)
    wpack = nc.dram_tensor("wpack", [128, 4 * D], f16, kind="ExternalInput")
    bpack = nc.dram_tensor("bpack", [128, 4], f32, kind="ExternalInput")
    # column-sharded output: yT[p, t] = out[token t, dim 128*core+p]
    yT = nc.dram_tensor("yT", [128, SQ], f32, kind="ExternalOutput")

    with tile.TileContext(nc) as tc:
        with (
            tc.tile_pool(name="const", bufs=1) as const,
            tc.tile_pool(name="psSc", bufs=3, space="PSUM") as psSc,
            tc.tile_pool(name="psCtx", bufs=2, space="PSUM") as psCtx,
            tc.tile_pool(name="psR", bufs=1, space="PSUM") as psR,
            tc.tile_pool(name="psO", bufs=2, space="PSUM") as psO,
            tc.tile_pool(name="pP", bufs=4) as pP,
            tc.tile_pool(name="pDen", bufs=2) as pDen,
            tc.tile_pool(name="pCg", bufs=2) as pCg,
            tc.tile_pool(name="pOut", bufs=2) as pOut,
            tc.tile_pool(name="dram", bufs=1, space="DRAM") as dram,
        ):
            # ---- constants / small DMAs first ----
            wpack_sb = const.tile([128, 4 * D], f16)
            nc.sync.dma_start(wpack_sb[:], wpack[:, :])
            bpack_sb = const.tile([128, 4], f32)
            nc.sync.dma_start(bpack_sb[:], bpack[:, :])
            wq_sb = wpack_sb[:, 0 * D:1 * D]
            wk_sb = wpack_sb[:, 1 * D:2 * D]
            wv_sb = wpack_sb[:, 2 * D:3 * D]
            wo_sb = wpack_sb[:, 3 * D:4 * D]  # [128, NKT*128] kt-major
            bq_sb = bpack_sb[:, 0:1]
            bk_sb = bpack_sb[:, 1:2]
            bvt_sb = bpack_sb[:, 2:3]
            bo_sb = bpack_sb[:, 3:4]

            ident = const.tile([128, 128], f16)
            make_identity(nc, ident)

            # eh weights: row h*64 ones, rest zero (selects that partition)
            e0 = const.tile([128, 128], f16)
            nc.vector.memset(e0[:], 0.0)
            nc.vector.memset(e0[0:1, :], 1.0)
            e1 = const.tile([128, 128], f16)
            nc.vector.memset(e1[:], 0.0)
            nc.vector.memset(e1[64:65, :], 1.0)
            esel = [e0, e1]

            # reciprocal staging: rows 0/1 get the two heads' denominators,
            # rows 2..127 stay 1.0 so recip_approx_fast stays finite there
            # (the op misbehaves on <128-partition APs).
            rec_st = const.tile([128, 512], f32, name="rec_st")
            nc.vector.memset(rec_st[:], 1.0)
            rec_full = const.tile([128, 512], f32, name="rec_full")
            rec16 = [const.tile([128, 512], f16, name=f"rec16{h}")
                     for h in range(HPC)]
            for h in range(HPC):
                nc.vector.memset(rec16[h][:], 0.0)

            # ---- xT load, segment-major so QKV can start early ----
            xt_sb = const.tile([128, NKT * SQ], f16)
            for seg in range(4):
                t0 = seg * 1024
                for kt in range(NKT):
                    nc.sync.dma_start(
                        xt_sb[:, kt * SQ + t0: kt * SQ + t0 + 1024],
                        xT[kt * 128:(kt + 1) * 128, t0:t0 + 1024],
                    )

            qT_sb = const.tile([128, SQ], f16)
            kT_sb = const.tile([128, SQ], f16)
            vT_sb = const.tile([128, SQ], f16)
            # V with ones column: per (b, head, key-tile) a [128, 65] region
            NREG = B * HPC * KT_S  # 64 regions
            vaug_sb = const.tile([128, NREG * 65], f16)
            ones_cols = vaug_sb.rearrange("p (r c) -> p r c", c=65)[:, :, 64:65]
            nc.vector.memset(ones_cols, 1.0)

            ctxT_sb = const.tile([128, SQ], f16)

            # warmup collective: absorb ncfw first-trigger init early
            wu_loc = dram.tile([128, 2], f16, name="wu_loc")
            nc.sync.dma_start(wu_loc[:], ctxT_sb[:, 0:2])
            wu_g = dram.tile([NCORES * 128, 2], f16, addr_space="Shared",
                             name="wu_g")
            nc.gpsimd.collective_compute(
                "AllGather", mybir.AluOpType.bypass,
                replica_groups=[list(range(NCORES))],
                ins=[wu_loc.opt()], outs=[wu_g.opt()],
            )

            # ---- QKV helpers ----
            def emit_qkv_chunk(b, proj, ncx):
                w_sb, b_sb, dst = (
                    (wq_sb, bq_sb, qT_sb), (wk_sb, bk_sb, kT_sb),
                    (wv_sb, bvt_sb, vT_sb),
                )[proj]
                tok0 = b * S + ncx * 512
                ps = psO.tile([128, 512], f32, tag="a",
                              name=f"qkv_{b}_{proj}_{ncx}")
                for kt in range(NKT):
                    nc.tensor.matmul(
                        ps[:],
                        lhsT=w_sb[:, kt * 128:(kt + 1) * 128],
                        rhs=xt_sb[:, kt * SQ + tok0: kt * SQ + tok0 + 512],
                        start=(kt == 0),
                        stop=(kt == NKT - 1),
                    )
                nc.scalar.activation(dst[:, tok0:tok0 + 512], ps[:],
                                     AF.Identity, bias=b_sb)

            def emit_vaug(b):
                # transpose V^T tiles into [tokens, dims] regions
                for tt in range(KT_S):
                    tok0 = b * S + tt * 128
                    vtr = psO.tile([128, 128], f16, tag="a",
                                   name=f"vtr_{b}_{tt}")
                    nc.tensor.transpose(vtr[:], vT_sb[:, tok0:tok0 + 128],
                                        ident[:])
                    for h in range(HPC):
                        r = (b * HPC + h) * KT_S + tt
                        nc.vector.tensor_copy(
                            vaug_sb[:, r * 65: r * 65 + 64],
                            vtr[:, h * 64:(h + 1) * 64],
                        )

            # ---- attention inner loop ----
            def emit_attention_qc(b, qc):
                q0 = b * S + qc * 512
                ctx_ps = [
                    psCtx.tile([65, 512], f32, tag="ctx",
                               name=f"ctx_{b}_{qc}_{h}")
                    for h in range(HPC)
                ]
                for kt in range(KT_S):
                    k0 = b * S + kt * 128
                    scs = []
                    for h in range(HPC):
                        sc = psSc.tile([128, 512], f32, tag="sc",
                                       name=f"sc_{b}_{qc}_{kt}_{h}")
                        nc.tensor.matmul(
                            sc[:],
                            lhsT=kT_sb[h * 64:(h + 1) * 64, k0:k0 + 128],
                            rhs=qT_sb[h * 64:(h + 1) * 64, q0:q0 + 512],
                            start=True, stop=True,
                            tile_position=(h * 64, 0),
                        )
                        scs.append(sc)
                    p_sb = pP.tile([128, 1024], f16, tag="p",
                                   name=f"p_{b}_{qc}_{kt}")
                    for h in range(HPC):
                        use_dve = (h == 1 and kt in _DVE_H1_KT)
                        if use_dve:
                            nc.vector.tensor_scalar(
                                p_sb.bitcast(i16)[:, h * 512:(h + 1) * 512],
                                scs[h][:], _SCH_A, _SCH_B,
                                ALU.mult, ALU.add,
                            )
                        else:
                            nc.scalar.activation(
                                p_sb[:, h * 512:(h + 1) * 512], scs[h][:],
                                AF.Exp, scale=0.125,
                            )
                    for h in range(HPC):
                        r = (b * HPC + h) * KT_S + kt
                        nc.tensor.matmul(
                            ctx_ps[h][:],
                            lhsT=vaug_sb[:, r * 65:(r + 1) * 65],
                            rhs=p_sb[:, h * 512:(h + 1) * 512],
                            start=(kt == 0),
                            stop=(kt == KT_S - 1),
                        )
                # normalize: denom row 64 -> recip -> broadcast -> multiply
                rps = psR.tile([128, 512], f32, tag="rps", name=f"rps_{b}_{qc}")
                for h in range(HPC):
                    nc.vector.tensor_copy(rec_st[h * 64:h * 64 + 1, :],
                                          ctx_ps[h][64:65, :])
                nc.vector.reciprocal_approx_fast(rec_full[:], rec_st[:])
                for h in range(HPC):
                    nc.vector.tensor_copy(rec16[h][h * 64:h * 64 + 1, :],
                                          rec_full[h * 64:h * 64 + 1, :])
                    nc.tensor.matmul(
                        rps[h * 64:(h + 1) * 64, :],
                        lhsT=esel[h][:, 0:64],
                        rhs=rec16[h][:],
                        start=True, stop=True,
                    )
                for h in range(HPC):
                    rps_sb = pDen.tile([64, 512], f16, tag="d",
                                       name=f"rd_{b}_{qc}_{h}")
                    nc.scalar.copy(rps_sb[:], rps[h * 64:(h + 1) * 64, :])
                    nc.vector.tensor_tensor(
                        ctxT_sb[h * 64:(h + 1) * 64, q0:q0 + 512],
                        ctx_ps[h][0:64, :], rps_sb[:], ALU.mult,
                    )

            # ---- per-qc AllGather + column-sharded output projection ----
            gath = {}

            def emit_gather(b, qc):
                q0 = b * S + qc * 512
                g_in = dram.tile([128, 512], f16, name=f"gin_{b}_{qc}")
                nc.sync.dma_start(g_in[:], ctxT_sb[:, q0:q0 + 512])
                g_out = dram.tile([NCORES * 128, 512], f16,
                                  addr_space="Shared", name=f"gout_{b}_{qc}")
                nc.gpsimd.collective_compute(
                    "AllGather", mybir.AluOpType.bypass,
                    replica_groups=[list(range(NCORES))],
                    ins=[g_in.opt()], outs=[g_out.opt()],
                )
                gath[(b, qc)] = g_out

            def emit_outproj(key):
                b, qc = key
                q0 = b * S + qc * 512
                g_out = gath[key]
                cg = pCg.tile([128, NKT, 512], f16, tag="cg",
                              name=f"cg_{b}_{qc}")
                for k in range(NKT):
                    nc.sync.dma_start(cg[:, k, :],
                                      g_out[k * 128:(k + 1) * 128, :])
                po = psO.tile([128, 512], f32, tag="a", name=f"po_{b}_{qc}")
                for k in range(NKT):
                    nc.tensor.matmul(
                        po[:],
                        lhsT=wo_sb[:, k * 128:(k + 1) * 128],
                        rhs=cg[:, k, :],
                        start=(k == 0),
                        stop=(k == NKT - 1),
                    )
                out_sb = pOut.tile([128, 512], f32, tag="os",
                                   name=f"os_{b}_{qc}")
                nc.vector.tensor_scalar(out_sb[:], po[:], bo_sb, None,
                                        ALU.add)
                nc.sync.dma_start(yT[:, q0:q0 + 512], out_sb[:])

            # ---- schedule ----
            # QKV b0 fully, then b0 attention with b1's QKV interleaved.
            for proj in range(3):
                for ncx in range(QC_S):
                    emit_qkv_chunk(0, proj, ncx)
            emit_vaug(0)

            # b1 QKV chunks to interleave, V first so vaug(1) can go early
            b1_chunks = ([(2, n) for n in range(QC_S)]
                         + [(0, n) for n in range(QC_S)]
                         + [(1, n) for n in range(QC_S)])
            pending_outproj = []
            for qc in range(QC_S):
                emit_attention_qc(0, qc)
                emit_gather(0, qc)
                pending_outproj.append((0, qc))
                for _ in range(3):
                    if b1_chunks:
                        p_, n_ = b1_chunks.pop(0)
                        emit_qkv_chunk(1, p_, n_)
                if qc == 2:
                    emit_vaug(1)
                if len(pending_outproj) > 1:
                    emit_outproj(pending_outproj.pop(0))
            for qc in range(QC_S):
                emit_attention_qc(1, qc)
                emit_gather(1, qc)
                pending_outproj.append((1, qc))
                if len(pending_outproj) > 1:
                    emit_outproj(pending_outproj.pop(0))
            for key in pending_outproj:
                emit_outproj(key)

    nc.finalize()
    return nc


def kernel(x, Wq, Wk, Wv, bq, bk, bv, Wo, bo):
    from concourse.bass_utils import run_bass_kernel_spmd

    if "nc" not in _CACHE:
        _CACHE["nc"] = _build_kernel()
    nc = _CACHE["nc"]

    # host-side prep
    xTh = np.ascontiguousarray(
        x.astype(np.float32).transpose(2, 0, 1).reshape(D, SQ)
    ).astype(np.float16)

    def pack_w(Wslice):
        # [D, M] -> [128, NKT*M] kt-major: out[p, kt*M+m] = Wslice[kt*128+p, m]
        M = Wslice.shape[1]
        return np.ascontiguousarray(
            Wslice.reshape(NKT, 128, M).transpose(1, 0, 2).reshape(128, NKT * M)
        ).astype(np.float16)

    in_maps = []
    for c in range(NCORES):
        hA, hB = HPC * c, HPC * c + 1
        wq_c = pack_w(np.concatenate([Wq[hA], Wq[hB]], axis=1))
        wk_c = pack_w(np.concatenate([Wk[hA], Wk[hB]], axis=1))
        wv_c = pack_w(np.concatenate([Wv[hA], Wv[hB]], axis=1))
        wo_c = pack_w(Wo[:, 128 * c:128 * (c + 1)])
        wpack_c = np.ascontiguousarray(
            np.concatenate([wq_c, wk_c, wv_c, wo_c], axis=1))
        bq_c = np.concatenate([bq[hA], bq[hB]]).reshape(128, 1)
        bk_c = np.concatenate([bk[hA], bk[hB]]).reshape(128, 1)
        bv_c = np.concatenate([bv[hA], bv[hB]]).reshape(128, 1)
        bo_c = bo[128 * c:128 * (c + 1)].reshape(128, 1)
        bpack_c = np.ascontiguousarray(
            np.concatenate([bq_c, bk_c, bv_c, bo_c],
                           axis=1)).astype(np.float32)
        in_maps.append({"xT": xTh, "wpack": wpack_c, "bpack": bpack_c})

    res = run_bass_kernel_spmd(nc, in_maps, core_ids=list(range(NCORES)))
    _CACHE["last_result"] = res
    # assemble: core c's yT [128, SQ] are output columns 128c..128c+127
    out = np.empty((B, S, D), dtype=np.float32)
    for c in range(NCORES):
        yt = res.results[c]["yT"]  # [128, SQ]
        out[:, :, 128 * c:128 * (c + 1)] = (
            yt.reshape(128, B, S).transpose(1, 2, 0)
        )
    return out
